# revision 22
# baseline (speedup 1.0000x reference)
"""BlockGrouper (MoE routing dispatch) Trainium2 kernel — raw bass.

Semantics (from the reference): each token n in sample b belongs to group
g = argmax(block_onehot[b, n]); its slot within the group is its rank
among same-group tokens in token order.  With the balanced one-hot
routing, the output [B, G, cap, D] is a pure row-permutation of
x [B, N, D].

Sharding: data-parallel over B across the 8 NeuronCores (one sample per
core); each core moves 16 MiB in + 16 MiB out (+ a 2 MiB scratch bounce
for the index inversion).

Why gather instead of scatter: dma_scatter_add's CCE-add makes the SDMA
engines read-modify-write every 2 KiB destination row (measured 173 ns
per descriptor vs 91 ns roofline), capping the scatter at ~190 GB/s.
dma_gather reads run at full line rate (~358 GB/s measured), and the
output can then be written with contiguous HWDGE stores (~390 GB/s).
The price is computing the INVERSE permutation src = dest^-1 on device:
scatter token ids into a 256 B-strided DRAM table (tbl[dest[n]] = n,
8x1024-index elem_size=1 scatters, 13 us) and read it back.

Per-core program (N=8192, G=16, D=512, cap=512, P=128, C=64; token n
lives at (p = n // 64, c = n % 64); output slot j at (q = j // 64,
c = j % 64)):
  1. dest[n] = g*cap + rank(n) = sum_g onehot * (prefix_c + carry_p +
     g*cap - 1): 16 strided tensor_tensor_scan ops + one
     strict-upper-triangular-ones matmul (carry) + const-row matmul.
  2. fold dest -> destw int16 (SWDGE index layout: slot i of op k at
     partition i%16, col i//16, replicated across the 8 Q7 core groups)
     via 8 replicated-selection matmuls + one strided DVE cast-copy.
  3. 8x 1024-idx dma_scatter_add elem_size=1 (4 B payload = token id,
     256 B row stride): tbl[dest[n]][0] = n.
  4. readback chunk k: tbl rows {q*64 + c : c in [8k, 8k+8)} as
     [128, 8, 64] contiguous-per-partition load; DVE picks col 0 ->
     src_f[:, 8k:8k+8]; 8 small fold matmuls + DVE copy -> srcw slice.
  5. gather op k: 1024-idx dma_gather out[q, 8k+t] = x[src[q*64+8k+t]];
     store op k: contiguous [128, 8, 512] -> out rows q*64 + [8k, 8k+8).
  Ops > 1024 indices hang the SWDGE descriptor ring — keep 1024.
"""


import numpy as np

B, N, G, D = 8, 8192, 16, 512
CAP = N // G
P = 128
C = N // P
NCORES = 8
NCHUNK = 8
GCH = N // NCHUNK  # 1024
TW = 64            # tbl row width (f32) -> 256 B stride

_cached = None


def _build():
    import concourse.bass as bass
    import concourse.bacc as bacc
    import concourse.mybir as mybir

    f32 = mybir.dt.float32
    i16 = mybir.dt.int16

    nc = bacc.Bacc("TRN2", target_bir_lowering=False, debug=False,
                   num_devices=NCORES, num_swdge_queues=4)
    x_d = nc.dram_tensor("x", [N, D], f32, kind="ExternalInput")
    oh_d = nc.dram_tensor("oh", [N, G], f32, kind="ExternalInput")
    cst_big_d = nc.dram_tensor("cst_big", [P, 9 * P], f32,
                               kind="ExternalInput")
    cst_row_d = nc.dram_tensor("cst_row", [1, P + G + C], f32,
                               kind="ExternalInput")
    iota_d = nc.dram_tensor("iota", [P, C], f32, kind="ExternalInput")
    out_d = nc.dram_tensor("out", [N, D], f32, kind="ExternalOutput")
    tbl_d = nc.dram_tensor("tbl", [N, TW], f32, kind="ExternalOutput")
    dummy_d = nc.dram_tensor("lib_warm", [16, 64], f32, kind="ExternalOutput")

    cc = C // NCHUNK  # 8 columns per chunk

    from contextlib import ExitStack
    with ExitStack() as ctx:
        cst_big_t = ctx.enter_context(
            nc.sbuf_tensor("cst_big_t", [P, 9 * P], f32))
        cst_row_t = ctx.enter_context(
            nc.sbuf_tensor("cst_row_t", [1, P + G + C], f32))
        iota_t = ctx.enter_context(nc.sbuf_tensor("iota_t", [P, C], f32))
        oh_t = ctx.enter_context(nc.sbuf_tensor("oh_t", [P, C * G], f32))
        scan_t = ctx.enter_context(
            nc.sbuf_tensor("scan_t", [P, C * G], f32))
        s_t = ctx.enter_context(nc.sbuf_tensor("s_t", [P, C * G], f32))
        prod_t = ctx.enter_context(
            nc.sbuf_tensor("prod_t", [P, C * G], f32))
        dest_f = ctx.enter_context(nc.sbuf_tensor("dest_f", [P, C], f32))
        destw_t = ctx.enter_context(
            nc.sbuf_tensor("destw_t", [P, N // 16], i16))
        rbbuf_t = ctx.enter_context(
            nc.sbuf_tensor("rbbuf_t", [P, NCHUNK * cc * TW], f32))
        dummy_idx = ctx.enter_context(
            nc.sbuf_tensor("dummy_idx", [P, 64], i16))
        dummy_pay = ctx.enter_context(
            nc.sbuf_tensor("dummy_pay", [P, 8], f32))
        dummy_g = ctx.enter_context(nc.sbuf_tensor("dummy_g", [P, 64], f32))
        gt = ctx.enter_context(nc.sbuf_tensor("gt", [P, C * D], f32))
        # srcw_t deliberately allocated AFTER the 16 MiB gt, far from
        # destw_t: the Q7 cores stream destw_t during the id-scatters and
        # their dcache prefetch otherwise pulls in stale lines of the
        # adjacent not-yet-written srcw_t (observed as chunk-0 gathers
        # using a few stale indices).
        srcw_t = ctx.enter_context(
            nc.sbuf_tensor("srcw_t", [P, N // 16], i16))
        a_ps = ctx.enter_context(nc.psum_tensor("a_ps", [P, G], f32))
        ps_w = ctx.enter_context(nc.psum_tensor("ps_w", [P, C * 8], f32))
        s_const = ctx.enter_context(nc.semaphore("s_const"))
        s_oh = ctx.enter_context(nc.semaphore("s_oh"))
        s_rbsy = ctx.enter_context(nc.semaphore("s_rbsy"))
        s_rbsc = ctx.enter_context(nc.semaphore("s_rbsc"))
        s_scat = ctx.enter_context(nc.semaphore("s_scat"))
        s_gq = [ctx.enter_context(nc.semaphore(f"s_gq{i}"))
                for i in range(4)]
        s_stor = ctx.enter_context(nc.semaphore("s_stor"))
        s_dve = ctx.enter_context(nc.semaphore("s_dve"))
        s_pe = ctx.enter_context(nc.semaphore("s_pe"))

        su_t = cst_big_t[:, 0:P]
        # repsel_t[t]: [128, 128] with [t*16+q, m*16+q] = 1 — fold matmul
        repsel = [cst_big_t[:, (1 + t) * P:(2 + t) * P] for t in range(8)]
        ones_t = cst_row_t[:, 0:P]
        cst_t = cst_row_t[:, P:P + G]

        # ---------------- plain DMAs ----------------
        # oh split across both HWDGE rings (it gates everything);
        # constants + iota on the ACT ring.
        oh_src = oh_d[:].rearrange("(p c) g -> p (c g)", p=P)
        half = C * G // 2
        nc.sync.dma_start(
            out=oh_t[:, 0:half], in_=oh_src[:, 0:half]).then_inc(s_oh, 16)
        nc.scalar.dma_start(
            out=oh_t[:, half:C * G], in_=oh_src[:, half:C * G]).then_inc(
            s_oh, 16)
        nc.scalar.dma_start(out=cst_big_t[:], in_=cst_big_d[:]).then_inc(
            s_const, 16)
        nc.scalar.dma_start(out=cst_row_t[:], in_=cst_row_d[:]).then_inc(
            s_const, 16)
        nc.scalar.dma_start(out=iota_t[:], in_=iota_d[:]).then_inc(
            s_const, 16)

        # ---------------- DVE: index pipeline ----------------
        nc.vector.wait_ge(s_oh, 32)
        for g in range(G):
            ins = nc.vector.tensor_tensor_scan(
                out=scan_t[:, g::G], data0=oh_t[:, g::G],
                data1=oh_t[:, g::G], initial=0.0,
                op0=mybir.AluOpType.add, op1=mybir.AluOpType.bypass)
            if g == G - 1:
                ins.then_inc(s_dve, 1)
        nc.vector.wait_ge(s_pe, 1)
        a_bcast = a_ps[:].unsqueeze(1).to_broadcast([P, C, G])
        nc.vector.tensor_tensor(
            out=s_t[:].rearrange("p (c g) -> p c g", g=G),
            in0=scan_t[:].rearrange("p (c g) -> p c g", g=G),
            in1=a_bcast, op=mybir.AluOpType.add)
        nc.vector.tensor_tensor(out=prod_t[:], in0=oh_t[:], in1=s_t[:],
                                op=mybir.AluOpType.mult)
        nc.vector.tensor_reduce(
            out=dest_f[:],
            in_=prod_t[:].rearrange("p (c g) -> p c g", g=G),
            axis=mybir.AxisListType.X,
            op=mybir.AluOpType.add).then_inc(s_dve, 1)
        # fold1: destw int16 (after PE fold matmuls)
        nc.vector.wait_ge(s_pe, 2)
        nc.vector.tensor_copy(
            out=destw_t[:].rearrange("q (c t) -> q c t", t=8),
            in_=ps_w[:].rearrange("q (t c) -> q c t", c=C)).then_inc(
            s_dve, 1)

        # ---------------- PE ----------------
        nc.tensor.wait_ge(s_const, 32)
        nc.tensor.wait_ge(s_dve, 1)
        rowtot = scan_t[:, (C - 1) * G: C * G]
        nc.tensor.matmul(out=a_ps[:], lhsT=su_t, rhs=rowtot,
                         start=True, stop=False)
        nc.tensor.matmul(out=a_ps[:], lhsT=ones_t, rhs=cst_t,
                         start=False, stop=True).then_inc(s_pe, 1)
        nc.tensor.wait_ge(s_dve, 2)
        for t in range(8):
            ins = nc.tensor.matmul(out=ps_w[:, t * C:(t + 1) * C],
                                   lhsT=repsel[t],
                                   rhs=dest_f[:], start=True, stop=True)
        ins.then_inc(s_pe, 1)

        # ---------------- GPSIMD: warms + id-scatter ----------------
        nc.gpsimd.memset(dummy_idx[:], 0)
        nidx_reg = nc.gpsimd.to_reg(GCH)
        # 1024-idx elem-1 warm scatter: same instruction shape as the real
        # id-scatters so the first real one starts at steady-state
        # emission speed (cold first op measured 7.4 us vs 1.7 warm)
        nc.gpsimd.memset(dummy_pay[:], 0)
        with nc.allow_non_contiguous_dma(reason="warm scatter"):
            nc.gpsimd.dma_scatter_add(
                dummy_d[:][:, 0:1],
                dummy_pay[:].rearrange("p (c one) -> p c one", one=1),
                dummy_idx[:],
                GCH, nidx_reg, 1, elem_step=64).then_inc(s_scat, 16)

        nc.gpsimd.wait_ge(s_const, 48)  # iota payload loaded
        nc.gpsimd.wait_ge(s_dve, 3)  # destw ready
        for k in range(NCHUNK):
            with nc.allow_non_contiguous_dma(reason="256B-stride tbl"):
                nc.gpsimd.dma_scatter_add(
                    tbl_d[:][:, 0:1],
                    iota_t[:, k * cc:(k + 1) * cc].rearrange(
                        "p (c one) -> p c one", one=1),
                    destw_t[:, k * (GCH // 16):(k + 1) * (GCH // 16)],
                    GCH, nidx_reg, 1, elem_step=TW,
                    queue_num=k % 4).then_inc(s_scat, 16)

        # ---------------- readback + fold2 + gather + store ----------
        # rb chunk k: tbl rows {q*64 + c : c in [8k, 8k+8)} -> [128,8,64]
        gt3 = gt[:].rearrange("p (c d) -> p c d", d=D)
        tbl3 = tbl_d[:].rearrange("(q c) f -> q c f", q=P)
        rb3 = rbbuf_t[:].rearrange("q (kc f) -> q kc f", f=TW)
        nc.sync.wait_ge(s_scat, 16 * (NCHUNK + 1))
        nc.scalar.wait_ge(s_scat, 16 * (NCHUNK + 1))
        for k in range(NCHUNK):
            eng = nc.sync if k % 2 == 0 else nc.scalar
            sem = s_rbsy if k % 2 == 0 else s_rbsc
            eng.dma_start(
                out=rb3[:, k * cc:(k + 1) * cc, :],
                in_=tbl3[:, k * cc:(k + 1) * cc, :]).then_inc(sem, 16)

        out3 = out_d[:].rearrange("(q c) d -> q c d", q=P)
        # PE fold2: one batched fold, same 64-column matmul shape as
        # fold1, reading the table column 0 DIRECTLY as a strided rhs
        # view of the DMA-written rbbuf (tiny staged DVE copies were
        # intermittently stale when the PE read them).
        nc.tensor.wait_ge(s_rbsy, 16 * 4)
        nc.tensor.wait_ge(s_rbsc, 16 * 4)
        rbcol0 = rb3[:, :, 0:1].rearrange("q c f -> q (c f)")
        for t in range(8):
            ins = nc.tensor.matmul(out=ps_w[:, t * C:(t + 1) * C],
                                   lhsT=repsel[t],
                                   rhs=rbcol0, start=True, stop=True)
        ins.then_inc(s_pe, 1)
        # DVE: srcw full copy (inc 4)
        nc.vector.wait_ge(s_pe, 3)
        nc.vector.tensor_copy(
            out=srcw_t[:].rearrange("q (c t) -> q c t", t=8),
            in_=ps_w[:].rearrange("q (t c) -> q c t", c=C)).then_inc(
            s_dve, 1)
        # 1024-idx warm gather in the otherwise-idle readback window:
        # warms the gather emission path (cold first op measured 8.7 us
        # vs ~5.5 warm).  idx = destw slice 0 (valid row ids); target =
        # gt chunk 0, overwritten by the real gather 0 on the same queue
        # (per-engine FIFO makes the overwrite ordered).
        nc.gpsimd.dma_gather(
            gt3[:, 0:cc, :], x_d[:],
            destw_t[:, 0:GCH // 16],
            GCH, nidx_reg, D, queue_num=0).then_inc(s_gq[0], 16)
        nc.gpsimd.wait_ge(s_dve, 4)
        for k in range(NCHUNK):
            nc.gpsimd.dma_gather(
                gt3[:, k * cc:(k + 1) * cc, :],
                x_d[:],
                srcw_t[:, k * (GCH // 16):(k + 1) * (GCH // 16)],
                GCH, nidx_reg, D, queue_num=k % 4).then_inc(s_gq[k % 4], 16)
            # store op k (alternate rings); queue 0 also carries the
            # dummy warm gather (hence the +1)
            eng = nc.sync if k % 2 == 0 else nc.scalar
            eng.wait_ge(s_gq[k % 4],
                        16 * (k // 4 + (2 if k % 4 == 0 else 1)))
            eng.dma_start(
                out=out3[:, k * cc:(k + 1) * cc, :],
                in_=gt3[:, k * cc:(k + 1) * cc, :]).then_inc(s_stor, 16)

        nc.gpsimd.wait_ge(s_stor, 16 * NCHUNK)

    nc.compile()
    return nc


def _get_nc():
    global _cached
    if _cached is None:
        _cached = _build()
    return _cached


def _constants():
    su = np.triu(np.ones((P, P), np.float32), k=1)
    rs = []
    for t in range(8):
        m = np.zeros((P, P), np.float32)
        for mm in range(8):
            for q in range(16):
                m[t * 16 + q, mm * 16 + q] = 1.0
        rs.append(m)
    cst_big = np.concatenate([su] + rs, axis=1)
    ones_r = np.ones((1, P), np.float32)
    cst = (np.arange(G, dtype=np.float32) * CAP - 1.0).reshape(1, G)
    cvals = np.arange(C, dtype=np.float32).reshape(1, C)
    cst_row = np.concatenate([ones_r, cst, cvals], axis=1)
    iota = (np.arange(N, dtype=np.float32).reshape(P, C))
    return cst_big, cst_row, iota


def kernel(x, block_onehot, capacity):
    from concourse.bass_utils import run_bass_kernel_spmd

    x = np.ascontiguousarray(np.asarray(x, dtype=np.float32))
    oh = np.asarray(block_onehot, dtype=np.float32)
    if oh.ndim == 2:
        oh = np.broadcast_to(oh[None], (B,) + oh.shape)
    oh = np.ascontiguousarray(oh)
    assert x.shape == (B, N, D), x.shape
    assert oh.shape == (B, N, G), oh.shape
    assert int(capacity) == CAP, capacity
    nc = _get_nc()
    cst_big, cst_row, iota = _constants()
    in_maps = [
        {"x": x[b], "oh": oh[b], "cst_big": cst_big, "cst_row": cst_row,
         "iota": iota}
        for b in range(B)
    ]
    res = run_bass_kernel_spmd(nc, in_maps, core_ids=list(range(NCORES)))
    return np.stack([res.results[b]["out"].reshape(G, CAP, D)
                     for b in range(B)])


# revision 24
# speedup vs baseline: 1.9582x; 1.9582x over previous
"""BlockGrouper (MoE routing dispatch) Trainium2 kernel — raw bass.

Semantics (from the reference): each token n in sample b belongs to group
g = argmax(block_onehot[b, n]); its slot within the group is its rank
among same-group tokens in token order.  With the balanced one-hot
routing, the output [B, G, cap, D] is a pure row-permutation of
x [B, N, D].

Sharding: data-parallel over B across the 8 NeuronCores (one sample per
core); each core moves 16 MiB in + 16 MiB out (+ a 2 MiB scratch bounce
for the index inversion).

Why gather instead of scatter: dma_scatter_add's CCE-add makes the SDMA
engines read-modify-write every 2 KiB destination row (measured 173 ns
per descriptor vs 91 ns roofline), capping the scatter at ~190 GB/s.
dma_gather reads run at full line rate (~358 GB/s measured), and the
output can then be written with contiguous HWDGE stores (~390 GB/s).
The price is computing the INVERSE permutation src = dest^-1 on device:
scatter token ids into a 256 B-strided DRAM table (tbl[dest[n]] = n,
8x1024-index elem_size=1 scatters, 13 us) and read it back.

Per-core program (N=8192, G=16, D=512, cap=512, P=128, C=64; token n
lives at (p = n // 64, c = n % 64); output slot j at (q = j // 64,
c = j % 64)):
  1. dest[n] = g*cap + rank(n) = sum_g onehot * (prefix_c + carry_p +
     g*cap - 1): 16 strided tensor_tensor_scan ops + one
     strict-upper-triangular-ones matmul (carry) + const-row matmul.
  2. fold dest -> destw int16 (SWDGE index layout: slot i of op k at
     partition i%16, col i//16, replicated across the 8 Q7 core groups)
     via 8 replicated-selection matmuls + one strided DVE cast-copy.
  3. 8x 1024-idx dma_scatter_add elem_size=1 (4 B payload = token id,
     256 B row stride): tbl[dest[n]][0] = n.
  4. readback chunk k: tbl rows {q*64 + c : c in [8k, 8k+8)} as
     [128, 8, 64] contiguous-per-partition load; DVE picks col 0 ->
     src_f[:, 8k:8k+8]; 8 small fold matmuls + DVE copy -> srcw slice.
  5. gather op k: 1024-idx dma_gather out[q, 8k+t] = x[src[q*64+8k+t]];
     store op k: contiguous [128, 8, 512] -> out rows q*64 + [8k, 8k+8).
  Ops > 1024 indices hang the SWDGE descriptor ring — keep 1024.
"""


import numpy as np

B, N, G, D = 8, 8192, 16, 512
CAP = N // G
P = 128
C = N // P
NCORES = 8
NCHUNK = 8
GCH = N // NCHUNK  # 1024
TW = 64            # tbl row width (f32) -> 256 B stride

_cached = None


def _build():
    import concourse.bass as bass
    import concourse.bacc as bacc
    import concourse.mybir as mybir

    f32 = mybir.dt.float32
    i16 = mybir.dt.int16

    nc = bacc.Bacc("TRN2", target_bir_lowering=False, debug=False,
                   num_devices=NCORES, num_swdge_queues=4)
    x_d = nc.dram_tensor("x", [N, D], f32, kind="ExternalInput")
    oh_d = nc.dram_tensor("oh", [N, G], f32, kind="ExternalInput")
    cst_big_d = nc.dram_tensor("cst_big", [P, 9 * P], f32,
                               kind="ExternalInput")
    cst_row_d = nc.dram_tensor("cst_row", [1, P + G + C], f32,
                               kind="ExternalInput")
    iota_d = nc.dram_tensor("iota", [P, C], f32, kind="ExternalInput")
    warm_idx_d = nc.dram_tensor("warm_idx", [P, 8], i16,
                                kind="ExternalInput")
    out_d = nc.dram_tensor("out", [N, D], f32, kind="ExternalOutput")
    tbl_d = nc.dram_tensor("tbl", [N, TW], f32, kind="ExternalOutput")
    dummy_d = nc.dram_tensor("lib_warm", [128, 64], f32,
                             kind="ExternalOutput")

    cc = C // NCHUNK  # 8 columns per chunk

    from contextlib import ExitStack
    with ExitStack() as ctx:
        cst_big_t = ctx.enter_context(
            nc.sbuf_tensor("cst_big_t", [P, 9 * P], f32))
        cst_row_t = ctx.enter_context(
            nc.sbuf_tensor("cst_row_t", [1, P + G + C], f32))
        iota_t = ctx.enter_context(nc.sbuf_tensor("iota_t", [P, C], f32))
        oh_t = ctx.enter_context(nc.sbuf_tensor("oh_t", [P, C * G], f32))
        scan_t = ctx.enter_context(
            nc.sbuf_tensor("scan_t", [P, C * G], f32))
        s_t = ctx.enter_context(nc.sbuf_tensor("s_t", [P, C * G], f32))
        prod_t = ctx.enter_context(
            nc.sbuf_tensor("prod_t", [P, C * G], f32))
        dest_f = ctx.enter_context(nc.sbuf_tensor("dest_f", [P, C], f32))
        destw_t = ctx.enter_context(
            nc.sbuf_tensor("destw_t", [P, N // 16], i16))
        rbbuf_t = ctx.enter_context(
            nc.sbuf_tensor("rbbuf_t", [P, NCHUNK * cc * TW], f32))
        warm_idx_t = ctx.enter_context(
            nc.sbuf_tensor("warm_idx_t", [P, 8], i16))
        dummy_pay = ctx.enter_context(
            nc.sbuf_tensor("dummy_pay", [P, 8], f32))
        dummy_g = ctx.enter_context(nc.sbuf_tensor("dummy_g", [P, 64], f32))
        gt = ctx.enter_context(nc.sbuf_tensor("gt", [P, C * D], f32))
        # srcw_t deliberately allocated AFTER the 16 MiB gt, far from
        # destw_t: the Q7 cores stream destw_t during the id-scatters and
        # their dcache prefetch otherwise pulls in stale lines of the
        # adjacent not-yet-written srcw_t (observed as chunk-0 gathers
        # using a few stale indices).
        srcw_t = ctx.enter_context(
            nc.sbuf_tensor("srcw_t", [P, N // 16], i16))
        a_ps = ctx.enter_context(nc.psum_tensor("a_ps", [P, G], f32))
        ps_w = ctx.enter_context(nc.psum_tensor("ps_w", [P, C * 8], f32))
        s_const = ctx.enter_context(nc.semaphore("s_const"))
        s_oh = ctx.enter_context(nc.semaphore("s_oh"))
        s_rbsy = ctx.enter_context(nc.semaphore("s_rbsy"))
        s_rbsc = ctx.enter_context(nc.semaphore("s_rbsc"))
        s_scat = ctx.enter_context(nc.semaphore("s_scat"))
        s_gq = [ctx.enter_context(nc.semaphore(f"s_gq{i}"))
                for i in range(4)]
        s_stor = ctx.enter_context(nc.semaphore("s_stor"))
        s_dve = ctx.enter_context(nc.semaphore("s_dve"))
        s_pe = ctx.enter_context(nc.semaphore("s_pe"))

        su_t = cst_big_t[:, 0:P]
        # repsel_t[t]: [128, 128] with [t*16+q, m*16+q] = 1 — fold matmul
        repsel = [cst_big_t[:, (1 + t) * P:(2 + t) * P] for t in range(8)]
        ones_t = cst_row_t[:, 0:P]
        cst_t = cst_row_t[:, P:P + G]

        # ---------------- plain DMAs ----------------
        # oh split across both HWDGE rings (it gates everything);
        # constants + iota on the ACT ring.
        oh_src = oh_d[:].rearrange("(p c) g -> p (c g)", p=P)
        half = C * G // 2
        nc.sync.dma_start(
            out=oh_t[:, 0:half], in_=oh_src[:, 0:half]).then_inc(s_oh, 16)
        nc.scalar.dma_start(
            out=oh_t[:, half:C * G], in_=oh_src[:, half:C * G]).then_inc(
            s_oh, 16)
        nc.scalar.dma_start(out=cst_big_t[:], in_=cst_big_d[:]).then_inc(
            s_const, 16)
        nc.scalar.dma_start(out=cst_row_t[:], in_=cst_row_d[:]).then_inc(
            s_const, 16)
        nc.scalar.dma_start(out=iota_t[:], in_=iota_d[:]).then_inc(
            s_const, 16)
        nc.sync.dma_start(out=warm_idx_t[:], in_=warm_idx_d[:]).then_inc(
            s_oh, 16)

        # ---------------- DVE: index pipeline ----------------
        # 48 = both oh halves + the warm_idx load (all inc s_oh)
        nc.vector.wait_ge(s_oh, 48)
        for g in range(G):
            ins = nc.vector.tensor_tensor_scan(
                out=scan_t[:, g::G], data0=oh_t[:, g::G],
                data1=oh_t[:, g::G], initial=0.0,
                op0=mybir.AluOpType.add, op1=mybir.AluOpType.bypass)
            if g == G - 1:
                ins.then_inc(s_dve, 1)
        nc.vector.wait_ge(s_pe, 1)
        a_bcast = a_ps[:].unsqueeze(1).to_broadcast([P, C, G])
        nc.vector.tensor_tensor(
            out=s_t[:].rearrange("p (c g) -> p c g", g=G),
            in0=scan_t[:].rearrange("p (c g) -> p c g", g=G),
            in1=a_bcast, op=mybir.AluOpType.add)
        nc.vector.tensor_tensor(out=prod_t[:], in0=oh_t[:], in1=s_t[:],
                                op=mybir.AluOpType.mult)
        nc.vector.tensor_reduce(
            out=dest_f[:],
            in_=prod_t[:].rearrange("p (c g) -> p c g", g=G),
            axis=mybir.AxisListType.X,
            op=mybir.AluOpType.add).then_inc(s_dve, 1)
        # fold1: destw int16 (after PE fold matmuls)
        nc.vector.wait_ge(s_pe, 2)
        nc.vector.tensor_copy(
            out=destw_t[:].rearrange("q (c t) -> q c t", t=8),
            in_=ps_w[:].rearrange("q (t c) -> q c t", c=C)).then_inc(
            s_dve, 1)

        # ---------------- PE ----------------
        nc.tensor.wait_ge(s_const, 32)
        nc.tensor.wait_ge(s_dve, 1)
        rowtot = scan_t[:, (C - 1) * G: C * G]
        nc.tensor.matmul(out=a_ps[:], lhsT=su_t, rhs=rowtot,
                         start=True, stop=False)
        nc.tensor.matmul(out=a_ps[:], lhsT=ones_t, rhs=cst_t,
                         start=False, stop=True).then_inc(s_pe, 1)
        nc.tensor.wait_ge(s_dve, 2)
        for t in range(8):
            ins = nc.tensor.matmul(out=ps_w[:, t * C:(t + 1) * C],
                                   lhsT=repsel[t],
                                   rhs=dest_f[:], start=True, stop=True)
        ins.then_inc(s_pe, 1)

        gt3 = gt[:].rearrange("p (c d) -> p c d", d=D)
        # ---------------- GPSIMD: warms + id-scatter ----------------
        nidx_reg = nc.gpsimd.to_reg(GCH)
        reg128 = nc.gpsimd.to_reg(128)
        nc.gpsimd.memset(dummy_pay[:], 0)
        # 128-idx warms at t=0 with DISTINCT target rows (colliding
        # CCE-adds serialize the RMW): warm the elem-1 scatter and the
        # 2 KiB gather emission paths so the first real ops run at
        # steady-state speed (cold firsts measured 7.4/8.7 us).
        nc.gpsimd.wait_ge(s_oh, 48)  # warm_idx loaded
        with nc.allow_non_contiguous_dma(reason="warm scatter"):
            nc.gpsimd.dma_scatter_add(
                dummy_d[:][:, 0:1],
                dummy_pay[:, 0:1].rearrange("p (c one) -> p c one", one=1),
                warm_idx_t[:], 128, reg128, 1,
                elem_step=64).then_inc(s_scat, 16)
        nc.gpsimd.dma_gather(
            gt3[:, 0:1, :], x_d[:], warm_idx_t[:], 128, reg128, D,
            queue_num=0).then_inc(s_gq[0], 16)

        nc.gpsimd.wait_ge(s_const, 48)  # iota payload loaded
        nc.gpsimd.wait_ge(s_dve, 3)  # destw ready
        for k in range(NCHUNK):
            with nc.allow_non_contiguous_dma(reason="256B-stride tbl"):
                nc.gpsimd.dma_scatter_add(
                    tbl_d[:][:, 0:1],
                    iota_t[:, k * cc:(k + 1) * cc].rearrange(
                        "p (c one) -> p c one", one=1),
                    destw_t[:, k * (GCH // 16):(k + 1) * (GCH // 16)],
                    GCH, nidx_reg, 1, elem_step=TW,
                    queue_num=k % 4).then_inc(s_scat, 16)

        # ---------------- readback + fold2 + gather + store ----------
        # rb chunk k: tbl rows {q*64 + c : c in [8k, 8k+8)} -> [128,8,64]
        tbl3 = tbl_d[:].rearrange("(q c) f -> q c f", q=P)
        rb3 = rbbuf_t[:].rearrange("q (kc f) -> q kc f", f=TW)
        nc.sync.wait_ge(s_scat, 16 * (NCHUNK + 1))
        nc.scalar.wait_ge(s_scat, 16 * (NCHUNK + 1))
        for k in range(NCHUNK):
            eng = nc.sync if k % 2 == 0 else nc.scalar
            sem = s_rbsy if k % 2 == 0 else s_rbsc
            eng.dma_start(
                out=rb3[:, k * cc:(k + 1) * cc, :],
                in_=tbl3[:, k * cc:(k + 1) * cc, :]).then_inc(sem, 16)

        out3 = out_d[:].rearrange("(q c) d -> q c d", q=P)
        # PE fold2: one batched fold, same 64-column matmul shape as
        # fold1, reading the table column 0 DIRECTLY as a strided rhs
        # view of the DMA-written rbbuf (tiny staged DVE copies were
        # intermittently stale when the PE read them).
        nc.tensor.wait_ge(s_rbsy, 16 * 4)
        nc.tensor.wait_ge(s_rbsc, 16 * 4)
        rbcol0 = rb3[:, :, 0:1].rearrange("q c f -> q (c f)")
        for t in range(8):
            ins = nc.tensor.matmul(out=ps_w[:, t * C:(t + 1) * C],
                                   lhsT=repsel[t],
                                   rhs=rbcol0, start=True, stop=True)
        ins.then_inc(s_pe, 1)
        # DVE: srcw full copy (inc 4)
        nc.vector.wait_ge(s_pe, 3)
        nc.vector.tensor_copy(
            out=srcw_t[:].rearrange("q (c t) -> q c t", t=8),
            in_=ps_w[:].rearrange("q (t c) -> q c t", c=C)).then_inc(
            s_dve, 1)
        nc.gpsimd.wait_ge(s_dve, 4)
        for k in range(NCHUNK):
            nc.gpsimd.dma_gather(
                gt3[:, k * cc:(k + 1) * cc, :],
                x_d[:],
                srcw_t[:, k * (GCH // 16):(k + 1) * (GCH // 16)],
                GCH, nidx_reg, D, queue_num=k % 4).then_inc(s_gq[k % 4], 16)
            # store op k (alternate rings); queue 0 also carries the
            # dummy warm gather (hence the +1)
            eng = nc.sync if k % 2 == 0 else nc.scalar
            eng.wait_ge(s_gq[k % 4],
                        16 * (k // 4 + (2 if k % 4 == 0 else 1)))
            eng.dma_start(
                out=out3[:, k * cc:(k + 1) * cc, :],
                in_=gt3[:, k * cc:(k + 1) * cc, :]).then_inc(s_stor, 16)

        nc.gpsimd.wait_ge(s_stor, 16 * NCHUNK)

    nc.compile()
    return nc


def _get_nc():
    global _cached
    if _cached is None:
        _cached = _build()
    return _cached


def _constants():
    su = np.triu(np.ones((P, P), np.float32), k=1)
    rs = []
    for t in range(8):
        m = np.zeros((P, P), np.float32)
        for mm in range(8):
            for q in range(16):
                m[t * 16 + q, mm * 16 + q] = 1.0
        rs.append(m)
    cst_big = np.concatenate([su] + rs, axis=1)
    ones_r = np.ones((1, P), np.float32)
    cst = (np.arange(G, dtype=np.float32) * CAP - 1.0).reshape(1, G)
    cvals = np.arange(C, dtype=np.float32).reshape(1, C)
    cst_row = np.concatenate([ones_r, cst, cvals], axis=1)
    iota = (np.arange(N, dtype=np.float32).reshape(P, C))
    wi = np.zeros((P, 8), np.int16)
    for m in range(8):
        wi[m * 16:(m + 1) * 16, :] = np.arange(128, dtype=np.int16
                                               ).reshape(8, 16).T
    return cst_big, cst_row, iota, wi


def kernel(x, block_onehot, capacity):
    from concourse.bass_utils import run_bass_kernel_spmd

    x = np.ascontiguousarray(np.asarray(x, dtype=np.float32))
    oh = np.asarray(block_onehot, dtype=np.float32)
    if oh.ndim == 2:
        oh = np.broadcast_to(oh[None], (B,) + oh.shape)
    oh = np.ascontiguousarray(oh)
    assert x.shape == (B, N, D), x.shape
    assert oh.shape == (B, N, G), oh.shape
    assert int(capacity) == CAP, capacity
    nc = _get_nc()
    cst_big, cst_row, iota, warm_idx = _constants()
    in_maps = [
        {"x": x[b], "oh": oh[b], "cst_big": cst_big, "cst_row": cst_row,
         "iota": iota, "warm_idx": warm_idx}
        for b in range(B)
    ]
    res = run_bass_kernel_spmd(nc, in_maps, core_ids=list(range(NCORES)))
    return np.stack([res.results[b]["out"].reshape(G, CAP, D)
                     for b in range(B)])
